# revision 7
# baseline (speedup 1.0000x reference)
import sys
sys.path.insert(0, '/opt/trn_rl_repo')
import numpy as np
import ml_dtypes

import concourse.bass as bass
import concourse.bacc as bacc
import concourse.tile as tile
from concourse import bass_utils, mybir
from concourse.masks import make_identity

BF16 = mybir.dt.bfloat16
F32 = mybir.dt.float32
I32 = mybir.dt.int32

N_CORES = 8
N_NODES = 10000
N_GRAPHS = 16
NUM_NEIGHBORS = 16
NPC = N_NODES // N_CORES          # 1250 nodes per core
NS = 1280                         # padded nodes per core (10 node tiles)
NT = NS // 128
NS8 = NS * N_CORES
INV = 1.0 / np.sqrt(NUM_NEIGHBORS)  # 0.25
DINS = [128, 144, 144, 144]
DOUTS = [144, 144, 144, 8]


def _silu(x):
    return x / (1.0 + np.exp(-x))


def _host_prep(node_input, node_attr, edge_attr, emb, params,
               edge_src, edge_dst, batch):
    layers = params['layers']
    wrad = []
    for p in layers[:4]:
        h = _silu(emb @ p['W1'])
        h = _silu(h @ p['W2'])
        wrad.append((h @ p['W3']).astype(np.float32))  # [E, do]

    order = np.argsort(edge_dst, kind='stable')
    core = {}
    span_max = 1
    for c in range(N_CORES):
        lo, hi = c * NPC, (c + 1) * NPC
        sel = order[(edge_dst[order] >= lo) & (edge_dst[order] < hi)]
        dl = edge_dst[sel] - lo
        tiles = [sel[(dl >= t * 128) & (dl < (t + 1) * 128)] for t in range(NT)]
        core[c] = (lo, tiles)
        span_max = max(span_max, max(len(t) for t in tiles))
    SPT = -(-span_max // 128)

    owner = edge_src // NPC
    src_pad = (owner * NS + (edge_src - owner * NPC)).astype(np.int32)

    idx = np.zeros((N_CORES, NT, 128, SPT), np.int32)
    ea_pack = np.zeros((N_CORES, NT, 128, SPT * 9), np.float32)
    w_pack = [np.zeros((N_CORES, NT, 128, SPT * d), np.float32) for d in DOUTS]
    swin = np.zeros((N_CORES, NT, 128, SPT * 128), ml_dtypes.bfloat16)
    for c in range(N_CORES):
        lo, tiles = core[c]
        for nt in range(NT):
            e = tiles[nt]
            k = len(e)
            t_of = np.arange(k) // 128
            p_of = np.arange(k) % 128
            idx[c, nt, p_of, t_of] = src_pad[e]
            for j in range(9):
                ea_pack[c, nt, p_of, t_of * 9 + j] = edge_attr[e, j]
            for li in range(4):
                d = DOUTS[li]
                for o in range(d):
                    w_pack[li][c, nt, p_of, t_of * d + o] = wrad[li][e, o]
            col = (edge_dst[e] - lo) - nt * 128
            swin[c, nt, p_of, t_of * 128 + col] = INV

    def reorder(W):  # [din, J, do] -> [K, do] matching z layout
        din, J, do = W.shape
        Wt = W.transpose(1, 0, 2)
        a = Wt[:, :min(128, din), :].reshape(J * min(128, din), do)
        if din > 128:
            b = Wt[:, 128:, :].reshape(J * (din - 128), do)
            return np.concatenate([a, b], 0)
        return a

    W2d = [reorder(layers[li]['Wtp']).astype(ml_dtypes.bfloat16) for li in range(4)]
    Wsc = [reorder(layers[li]['Wsc']).astype(ml_dtypes.bfloat16) for li in range(4)]
    Wf2d = reorder(params['Wf']).astype(ml_dtypes.bfloat16)  # [72, 8]

    xpad0 = np.zeros((NS8, 128), np.float32)
    nattr = np.zeros((N_CORES, NS, 4), np.float32)
    sc0 = np.zeros((N_CORES, NS, 144), np.float32)
    bgr = np.zeros((N_CORES, NS, 16), ml_dtypes.bfloat16)
    sc0_full = np.einsum('ni,nj,ijo->no', node_input, node_attr,
                         layers[0]['Wsc']).astype(np.float32)
    for c in range(N_CORES):
        lo = c * NPC
        xpad0[c * NS: c * NS + NPC] = node_input[lo:lo + NPC]
        nattr[c, :NPC] = node_attr[lo:lo + NPC]
        sc0[c, :NPC] = sc0_full[lo:lo + NPC]
        bgr[c, np.arange(NPC), batch[lo:lo + NPC]] = INV

    return dict(SPT=SPT, idx=idx, ea=ea_pack, w=w_pack, swin=swin,
                W2d=W2d, Wsc=Wsc, Wf2d=Wf2d, xpad0=xpad0, nattr=nattr,
                sc0=sc0, bgr=bgr)


def _build(nc, SPT):
    t_in = {}
    t_in['idx'] = nc.dram_tensor("idx", [NT, 128, SPT], I32, kind="ExternalInput")
    t_in['ea'] = nc.dram_tensor("ea", [NT, 128, SPT * 9], F32, kind="ExternalInput")
    for li in range(4):
        d = DOUTS[li]
        t_in[f'w{li}'] = nc.dram_tensor(f"w{li}", [NT, 128, SPT * d], F32,
                                        kind="ExternalInput")
        t_in[f'W2d{li}'] = nc.dram_tensor(f"W2d{li}", [DINS[li] * 9, d], BF16,
                                          kind="ExternalInput")
        t_in[f'Wsc{li}'] = nc.dram_tensor(f"Wsc{li}", [DINS[li] * 4, d], BF16,
                                          kind="ExternalInput")
    t_in['swin'] = nc.dram_tensor("swin", [NT, 128, SPT * 128], BF16,
                                  kind="ExternalInput")
    t_in['Wf2d'] = nc.dram_tensor("Wf2d", [72, 8], BF16, kind="ExternalInput")
    t_in['xpad0'] = nc.dram_tensor("xpad0", [NS8, 128], F32, kind="ExternalInput")
    t_in['nattr'] = nc.dram_tensor("nattr", [NS, 4], F32, kind="ExternalInput")
    t_in['sc0'] = nc.dram_tensor("sc0", [NS, 144], F32, kind="ExternalInput")
    t_in['bgr'] = nc.dram_tensor("bgr", [NS, 16], BF16, kind="ExternalInput")
    out = nc.dram_tensor("out", [16, 8], F32, kind="ExternalOutput")

    with tile.TileContext(nc) as tc:
        with tc.tile_pool(name="const", bufs=1) as constp, \
             tc.tile_pool(name="wpool", bufs=1) as wpool, \
             tc.tile_pool(name="sup", bufs=2) as sup, \
             tc.tile_pool(name="zb", bufs=2) as zb, \
             tc.tile_pool(name="zt", bufs=2) as ztp, \
             tc.tile_pool(name="small", bufs=3) as sm, \
             tc.tile_pool(name="nodes", bufs=2) as ndp, \
             tc.tile_pool(name="pz", bufs=2, space="PSUM") as pz, \
             tc.tile_pool(name="pm", bufs=2, space="PSUM") as pm, \
             tc.tile_pool(name="pagg", bufs=1, space="PSUM") as pagg, \
             tc.tile_pool(name="pout", bufs=1, space="PSUM") as pout, \
             tc.tile_pool(name="dram", bufs=1, space="DRAM") as dram:

            ident = constp.tile([128, 128], BF16)
            make_identity(nc, ident[:])

            def load_wchunks(name, K, d):
                nchunk = -(-K // 128)
                t = wpool.tile([128, nchunk * d], BF16, tag=name, name="w_" + name)
                for ci in range(nchunk):
                    r0, r1 = ci * 128, min(K, (ci + 1) * 128)
                    nc.sync.dma_start(out=t[0:r1 - r0, ci * d:(ci + 1) * d],
                                      in_=t_in[name][r0:r1, :])
                return t

            Wz = {li: load_wchunks(f'W2d{li}', DINS[li] * 9, DOUTS[li])
                  for li in range(4)}
            Ws = {li: load_wchunks(f'Wsc{li}', DINS[li] * 4, DOUTS[li])
                  for li in range(4)}
            Wf_t = wpool.tile([72, 8], BF16)
            nc.sync.dma_start(out=Wf_t[:], in_=t_in['Wf2d'][:, :])
            bgr_t = wpool.tile([128, NT * 16], BF16)
            nattr_t = wpool.tile([128, NT * 4], F32)
            for nt in range(NT):
                nc.sync.dma_start(out=bgr_t[:, nt * 16:(nt + 1) * 16],
                                  in_=t_in['bgr'][nt * 128:(nt + 1) * 128, :])
                nc.sync.dma_start(out=nattr_t[:, nt * 4:(nt + 1) * 4],
                                  in_=t_in['nattr'][nt * 128:(nt + 1) * 128, :])

            xloc = [dram.tile([NS, d], F32, tag=f"xloc{i}", name=f"xloc{i}")
                    for i, d in enumerate(DOUTS)]
            xg = [dram.tile([NS8, d], F32, tag=f"xg{i}", name=f"xg{i}", addr_space="Shared")
                  for i, d in enumerate(DOUTS)]
            out_ps = pout.tile([16, 8], F32, space="PSUM")

            def mul_bcast(zt, zsl, xs_ap, ea_t, nj, blk):
                for j in range(nj):
                    nc.vector.tensor_tensor(
                        out=zt[:, zsl + j * blk: zsl + (j + 1) * blk],
                        in0=xs_ap,
                        in1=ea_t[:, j:j + 1].to_broadcast([128, blk]),
                        op=mybir.AluOpType.mult)

            def transpose_to(zT, zsrc, K):
                nchunk = -(-K // 128)
                for g in range(-(-nchunk // 4)):
                    c0, c1 = g * 4, min(nchunk, (g + 1) * 4)
                    pzt = pz.tile([128, 512], BF16, space="PSUM", tag="pz")
                    for ci in range(c0, c1):
                        r = min(128, K - ci * 128)
                        nc.tensor.transpose(
                            out=pzt[0:r, (ci - c0) * 128:(ci - c0) * 128 + 128],
                            in_=zsrc[:, ci * 128: ci * 128 + r],
                            identity=ident[:])
                    nc.scalar.copy(out=zT[:, c0 * 128:c1 * 128],
                                   in_=pzt[:, 0:(c1 - c0) * 128])

            def edge_layer(li, x_tab, last=False):
                din, do = (8, 8) if last else (DINS[li], DOUTS[li])
                K = din * 9
                nchunk = -(-K // 128)
                for nt in range(NT):
                    agg = pagg.tile([128, do], F32, space="PSUM", tag="agg")
                    idx_t = sm.tile([128, SPT], I32, tag="idx")
                    nc.sync.dma_start(out=idx_t[:], in_=t_in['idx'][nt, :, :])
                    xs = sup.tile([128, SPT * din], F32, tag="xs")
                    for s in range(SPT):
                        nc.gpsimd.indirect_dma_start(
                            out=xs[:, s * din:(s + 1) * din], out_offset=None,
                            in_=x_tab[:, :],
                            in_offset=bass.IndirectOffsetOnAxis(
                                ap=idx_t[:, s:s + 1], axis=0))
                    ea_s = sup.tile([128, SPT * 9], F32, tag="eas")
                    nc.sync.dma_start(out=ea_s[:], in_=t_in['ea'][nt, :, :])
                    sw_s = sup.tile([128, SPT * 128], BF16, tag="sws")
                    nc.sync.dma_start(out=sw_s[:], in_=t_in['swin'][nt, :, :])
                    if not last:
                        w_s = sup.tile([128, SPT * do], F32, tag="ws")
                        nc.sync.dma_start(out=w_s[:], in_=t_in[f'w{li}'][nt, :, :])
                    for t in range(SPT):
                        ea_t = ea_s[:, t * 9:(t + 1) * 9]
                        xs_t = xs[:, t * din:(t + 1) * din]
                        z = zb.tile([128, nchunk * 128], BF16, tag="z")
                        if din > 128:
                            mul_bcast(z, 0, xs_t[:, 0:128], ea_t, 9, 128)
                            mul_bcast(z, 9 * 128, xs_t[:, 128:din], ea_t, 9,
                                      din - 128)
                        else:
                            mul_bcast(z, 0, xs_t[:, 0:din], ea_t, 9, din)
                        zT = ztp.tile([128, nchunk * 128], BF16, tag="zT")
                        transpose_to(zT, z, K)
                        msg = pm.tile([128, do], F32, space="PSUM", tag="msg")
                        for ci in range(nchunk):
                            r = min(128, K - ci * 128)
                            nc.tensor.matmul(
                                out=msg[:],
                                lhsT=zT[0:r, ci * 128:ci * 128 + 128],
                                rhs=(Wf_t[0:r, :] if last
                                     else Wz[li][0:r, ci * do:(ci + 1) * do]),
                                start=(ci == 0), stop=(ci == nchunk - 1))
                        mw = sm.tile([128, do], BF16, tag="mw")
                        if last:
                            nc.vector.tensor_copy(out=mw[:], in_=msg[:])
                        else:
                            nc.vector.tensor_tensor(
                                out=mw[:], in0=msg[:],
                                in1=w_s[:, t * do:(t + 1) * do],
                                op=mybir.AluOpType.mult)
                        sc_more = (not last) and li > 0
                        nc.tensor.matmul(
                            out=agg[:], lhsT=sw_s[:, t * 128:(t + 1) * 128],
                            rhs=mw[:], start=(t == 0),
                            stop=(t == SPT - 1 and not sc_more))
                    # ---- finalize node tile ----
                    if last:
                        a4 = sm.tile([128, 8], BF16, tag="a4")
                        nc.vector.tensor_copy(out=a4[:], in_=agg[:])
                        nc.tensor.matmul(
                            out=out_ps[:], lhsT=bgr_t[:, nt * 16:(nt + 1) * 16],
                            rhs=a4[:], start=(nt == 0), stop=(nt == NT - 1))
                        continue
                    xn = ndp.tile([128, do], F32, tag="xn")
                    if li == 0:
                        sct = ndp.tile([128, do], F32, tag="sct")
                        nc.sync.dma_start(
                            out=sct[:], in_=t_in['sc0'][nt * 128:(nt + 1) * 128, :])
                        nc.vector.tensor_tensor(out=xn[:], in0=agg[:], in1=sct[:],
                                                op=mybir.AluOpType.add)
                        nc.scalar.activation(
                            out=xn[:], in_=xn[:],
                            func=mybir.ActivationFunctionType.Silu)
                    else:
                        xloc_t = ndp.tile([128, din], F32, tag="xloc_t")
                        nc.sync.dma_start(
                            out=xloc_t[:],
                            in_=xloc[li - 1][nt * 128:(nt + 1) * 128, :])
                        Ksc = din * 4
                        nsck = -(-Ksc // 128)
                        zs = zb.tile([128, nsck * 128], BF16, tag="zsc")
                        na = nattr_t[:, nt * 4:(nt + 1) * 4]
                        if din > 128:
                            mul_bcast(zs, 0, xloc_t[:, 0:128], na, 4, 128)
                            mul_bcast(zs, 4 * 128, xloc_t[:, 128:din], na, 4,
                                      din - 128)
                        else:
                            mul_bcast(zs, 0, xloc_t[:, 0:din], na, 4, din)
                        zsT = ztp.tile([128, nsck * 128], BF16, tag="zscT")
                        transpose_to(zsT, zs, Ksc)
                        for ci in range(nsck):
                            r = min(128, Ksc - ci * 128)
                            nc.tensor.matmul(
                                out=agg[:],
                                lhsT=zsT[0:r, ci * 128:ci * 128 + 128],
                                rhs=Ws[li][0:r, ci * do:(ci + 1) * do],
                                start=False, stop=(ci == nsck - 1))
                        if li < 3:
                            nc.scalar.activation(
                                out=xn[:], in_=agg[:],
                                func=mybir.ActivationFunctionType.Silu)
                        else:
                            nc.scalar.copy(out=xn[:], in_=agg[:])
                    nc.sync.dma_start(out=xloc[li][nt * 128:(nt + 1) * 128, :],
                                      in_=xn[:])
                if not last:
                    nc.gpsimd.collective_compute(
                        "AllGather", mybir.AluOpType.bypass,
                        replica_groups=[list(range(N_CORES))],
                        ins=[xloc[li].opt()], outs=[xg[li].opt()])

            edge_layer(0, t_in['xpad0'])
            edge_layer(1, xg[0])
            edge_layer(2, xg[1])
            edge_layer(3, xg[2])
            edge_layer(4, xg[3], last=True)

            ot = sm.tile([16, 8], F32, tag="ot")
            nc.vector.tensor_copy(out=ot[:], in_=out_ps[:])
            nc.sync.dma_start(out=out[:, :], in_=ot[:])
    return out


def kernel(**inputs):
    node_input = np.asarray(inputs['node_input'], np.float32)
    node_attr = np.asarray(inputs['node_attr'], np.float32)
    edge_attr = np.asarray(inputs['edge_attr'], np.float32)
    emb = np.asarray(inputs['edge_length_embedding'], np.float32)
    params = inputs['params']
    edge_src = np.asarray(inputs['edge_src']).astype(np.int64)
    edge_dst = np.asarray(inputs['edge_dst']).astype(np.int64)
    batch = np.asarray(inputs['batch']).astype(np.int64)

    pk = _host_prep(node_input, node_attr, edge_attr, emb, params,
                    edge_src, edge_dst, batch)

    nc = bacc.Bacc("TRN2", target_bir_lowering=False, debug=False,
                   num_devices=N_CORES)
    _build(nc, pk['SPT'])
    nc.compile()

    in_maps = []
    for c in range(N_CORES):
        m = dict(idx=pk['idx'][c], ea=pk['ea'][c], swin=np.asarray(pk['swin'][c]),
                 Wf2d=np.asarray(pk['Wf2d']), xpad0=pk['xpad0'],
                 nattr=pk['nattr'][c], sc0=pk['sc0'][c],
                 bgr=np.asarray(pk['bgr'][c]))
        for li in range(4):
            m[f'w{li}'] = pk['w'][li][c]
            m[f'W2d{li}'] = np.asarray(pk['W2d'][li])
            m[f'Wsc{li}'] = np.asarray(pk['Wsc'][li])
        in_maps.append(m)

    import os, time as _time
    trace = os.environ.get("KERNEL_TRACE") == "1"
    _t0 = _time.time()
    res = bass_utils.run_bass_kernel_spmd(nc, in_maps,
                                          core_ids=list(range(N_CORES)),
                                          trace=trace)
    kernel.last_run_s = _time.time() - _t0
    out = np.zeros((16, 8), np.float32)
    for c in range(N_CORES):
        out += res.results[c]['out']
    kernel.last_results = res
    return out


# revision 8
# speedup vs baseline: 1.1425x; 1.1425x over previous
import sys
sys.path.insert(0, '/opt/trn_rl_repo')
import numpy as np
import ml_dtypes

import concourse.bass as bass
import concourse.bacc as bacc
import concourse.tile as tile
from concourse import bass_utils, mybir
from concourse.masks import make_identity

BF16 = mybir.dt.bfloat16
F32 = mybir.dt.float32
I32 = mybir.dt.int32

N_CORES = 8
N_NODES = 10000
N_GRAPHS = 16
NUM_NEIGHBORS = 16
NPC = N_NODES // N_CORES          # 1250 nodes per core
NS = 1280                         # padded nodes per core (10 node tiles)
NT = NS // 128
NS8 = NS * N_CORES
INV = 1.0 / np.sqrt(NUM_NEIGHBORS)  # 0.25
DINS = [128, 144, 144, 144]
DOUTS = [144, 144, 144, 8]


def _silu(x):
    return x / (1.0 + np.exp(-x))


def _host_prep(node_input, node_attr, edge_attr, emb, params,
               edge_src, edge_dst, batch):
    layers = params['layers']
    wrad = []
    for p in layers[:4]:
        h = _silu(emb @ p['W1'])
        h = _silu(h @ p['W2'])
        wrad.append((h @ p['W3']).astype(np.float32))  # [E, do]

    order = np.argsort(edge_dst, kind='stable')
    core = {}
    span_max = 1
    for c in range(N_CORES):
        lo, hi = c * NPC, (c + 1) * NPC
        sel = order[(edge_dst[order] >= lo) & (edge_dst[order] < hi)]
        dl = edge_dst[sel] - lo
        tiles = [sel[(dl >= t * 128) & (dl < (t + 1) * 128)] for t in range(NT)]
        core[c] = (lo, tiles)
        span_max = max(span_max, max(len(t) for t in tiles))
    SPT = -(-span_max // 128)

    owner = edge_src // NPC
    src_pad = (owner * NS + (edge_src - owner * NPC)).astype(np.int32)

    idx = np.zeros((N_CORES, NT, 128, SPT), np.int32)
    ea_pack = np.zeros((N_CORES, NT, 128, SPT * 9), np.float32)
    w_pack = [np.zeros((N_CORES, NT, 128, SPT * d), np.float32) for d in DOUTS]
    swin = np.zeros((N_CORES, NT, 128, SPT * 128), ml_dtypes.bfloat16)
    for c in range(N_CORES):
        lo, tiles = core[c]
        for nt in range(NT):
            e = tiles[nt]
            k = len(e)
            t_of = np.arange(k) // 128
            p_of = np.arange(k) % 128
            idx[c, nt, p_of, t_of] = src_pad[e]
            for j in range(9):
                ea_pack[c, nt, p_of, t_of * 9 + j] = edge_attr[e, j]
            for li in range(4):
                d = DOUTS[li]
                for o in range(d):
                    w_pack[li][c, nt, p_of, t_of * d + o] = wrad[li][e, o]
            col = (edge_dst[e] - lo) - nt * 128
            swin[c, nt, p_of, t_of * 128 + col] = INV

    def reorder(W):  # [din, J, do] -> [K, do] matching z layout
        din, J, do = W.shape
        Wt = W.transpose(1, 0, 2)
        a = Wt[:, :min(128, din), :].reshape(J * min(128, din), do)
        if din > 128:
            b = Wt[:, 128:, :].reshape(J * (din - 128), do)
            return np.concatenate([a, b], 0)
        return a

    W2d = [reorder(layers[li]['Wtp']).astype(ml_dtypes.bfloat16) for li in range(4)]
    Wsc = [reorder(layers[li]['Wsc']).astype(ml_dtypes.bfloat16) for li in range(4)]
    Wf2d = reorder(params['Wf']).astype(ml_dtypes.bfloat16)  # [72, 8]

    xpad0 = np.zeros((NS8, 128), np.float32)
    nattr = np.zeros((N_CORES, NS, 4), np.float32)
    sc0 = np.zeros((N_CORES, NS, 144), np.float32)
    bgr = np.zeros((N_CORES, NS, 16), ml_dtypes.bfloat16)
    sc0_full = np.einsum('ni,nj,ijo->no', node_input, node_attr,
                         layers[0]['Wsc']).astype(np.float32)
    for c in range(N_CORES):
        lo = c * NPC
        xpad0[c * NS: c * NS + NPC] = node_input[lo:lo + NPC]
        nattr[c, :NPC] = node_attr[lo:lo + NPC]
        sc0[c, :NPC] = sc0_full[lo:lo + NPC]
        bgr[c, np.arange(NPC), batch[lo:lo + NPC]] = INV

    return dict(SPT=SPT, idx=idx, ea=ea_pack, w=w_pack, swin=swin,
                W2d=W2d, Wsc=Wsc, Wf2d=Wf2d, xpad0=xpad0, nattr=nattr,
                sc0=sc0, bgr=bgr)


def _build(nc, SPT):
    t_in = {}
    t_in['idx'] = nc.dram_tensor("idx", [NT, 128, SPT], I32, kind="ExternalInput")
    t_in['ea'] = nc.dram_tensor("ea", [NT, 128, SPT * 9], F32, kind="ExternalInput")
    for li in range(4):
        d = DOUTS[li]
        t_in[f'w{li}'] = nc.dram_tensor(f"w{li}", [NT, 128, SPT * d], F32,
                                        kind="ExternalInput")
        t_in[f'W2d{li}'] = nc.dram_tensor(f"W2d{li}", [DINS[li] * 9, d], BF16,
                                          kind="ExternalInput")
        t_in[f'Wsc{li}'] = nc.dram_tensor(f"Wsc{li}", [DINS[li] * 4, d], BF16,
                                          kind="ExternalInput")
    t_in['swin'] = nc.dram_tensor("swin", [NT, 128, SPT * 128], BF16,
                                  kind="ExternalInput")
    t_in['Wf2d'] = nc.dram_tensor("Wf2d", [72, 8], BF16, kind="ExternalInput")
    t_in['xpad0'] = nc.dram_tensor("xpad0", [NS8, 128], F32, kind="ExternalInput")
    t_in['nattr'] = nc.dram_tensor("nattr", [NS, 4], F32, kind="ExternalInput")
    t_in['sc0'] = nc.dram_tensor("sc0", [NS, 144], F32, kind="ExternalInput")
    t_in['bgr'] = nc.dram_tensor("bgr", [NS, 16], BF16, kind="ExternalInput")
    out = nc.dram_tensor("out", [16, 8], F32, kind="ExternalOutput")

    with tile.TileContext(nc) as tc:
        with tc.tile_pool(name="const", bufs=1) as constp, \
             tc.tile_pool(name="wpool", bufs=1) as wpool, \
             tc.tile_pool(name="sup", bufs=3) as sup, \
             tc.tile_pool(name="zb", bufs=2) as zb, \
             tc.tile_pool(name="zt", bufs=2) as ztp, \
             tc.tile_pool(name="small", bufs=3) as sm, \
             tc.tile_pool(name="nodes", bufs=2) as ndp, \
             tc.tile_pool(name="pz", bufs=2, space="PSUM") as pz, \
             tc.tile_pool(name="pm", bufs=2, space="PSUM") as pm, \
             tc.tile_pool(name="pagg", bufs=1, space="PSUM") as pagg, \
             tc.tile_pool(name="pout", bufs=1, space="PSUM") as pout, \
             tc.tile_pool(name="dram", bufs=1, space="DRAM") as dram:

            ident = constp.tile([128, 128], BF16)
            make_identity(nc, ident[:])

            def load_wchunks(name, K, d):
                nchunk = -(-K // 128)
                t = wpool.tile([128, nchunk * d], BF16, tag=name, name="w_" + name)
                for ci in range(nchunk):
                    r0, r1 = ci * 128, min(K, (ci + 1) * 128)
                    nc.sync.dma_start(out=t[0:r1 - r0, ci * d:(ci + 1) * d],
                                      in_=t_in[name][r0:r1, :])
                return t

            Wz = {li: load_wchunks(f'W2d{li}', DINS[li] * 9, DOUTS[li])
                  for li in range(4)}
            Ws = {li: load_wchunks(f'Wsc{li}', DINS[li] * 4, DOUTS[li])
                  for li in range(4)}
            Wf_t = wpool.tile([72, 8], BF16)
            nc.sync.dma_start(out=Wf_t[:], in_=t_in['Wf2d'][:, :])
            bgr_t = wpool.tile([128, NT * 16], BF16)
            nattr_t = wpool.tile([128, NT * 4], F32)
            for nt in range(NT):
                nc.sync.dma_start(out=bgr_t[:, nt * 16:(nt + 1) * 16],
                                  in_=t_in['bgr'][nt * 128:(nt + 1) * 128, :])
                nc.sync.dma_start(out=nattr_t[:, nt * 4:(nt + 1) * 4],
                                  in_=t_in['nattr'][nt * 128:(nt + 1) * 128, :])

            xloc = [dram.tile([NS, d], F32, tag=f"xloc{i}", name=f"xloc{i}")
                    for i, d in enumerate(DOUTS)]
            xg = [dram.tile([NS8, d], F32, tag=f"xg{i}", name=f"xg{i}", addr_space="Shared")
                  for i, d in enumerate(DOUTS)]
            out_ps = pout.tile([16, 8], F32, space="PSUM")

            def mul_bcast(zt, zsl, xs_ap, ea_t, nj, blk):
                nc.vector.tensor_tensor(
                    out=zt[:, zsl:zsl + nj * blk].rearrange(
                        "p (j d) -> p j d", j=nj),
                    in0=xs_ap.rearrange("p (a d) -> p a d", a=1)
                        .to_broadcast([128, nj, blk]),
                    in1=ea_t.rearrange("p (j o) -> p j o", o=1)
                        .to_broadcast([128, nj, blk]),
                    op=mybir.AluOpType.mult)

            def transpose_to(zT, zsrc, K):
                nchunk = -(-K // 128)
                for g in range(-(-nchunk // 4)):
                    c0, c1 = g * 4, min(nchunk, (g + 1) * 4)
                    pzt = pz.tile([128, 512], BF16, space="PSUM", tag="pz")
                    for ci in range(c0, c1):
                        r = min(128, K - ci * 128)
                        nc.tensor.transpose(
                            out=pzt[0:r, (ci - c0) * 128:(ci - c0) * 128 + 128],
                            in_=zsrc[:, ci * 128: ci * 128 + r],
                            identity=ident[:])
                    nc.scalar.copy(out=zT[:, c0 * 128:c1 * 128],
                                   in_=pzt[:, 0:(c1 - c0) * 128])

            def edge_layer(li, x_tab, last=False):
                din, do = (8, 8) if last else (DINS[li], DOUTS[li])
                K = din * 9
                nchunk = -(-K // 128)
                for nt in range(NT):
                    agg = pagg.tile([128, do], F32, space="PSUM", tag="agg")
                    idx_t = sm.tile([128, SPT], I32, tag="idx")
                    nc.sync.dma_start(out=idx_t[:], in_=t_in['idx'][nt, :, :])
                    xs = sup.tile([128, SPT * din], F32, tag="xs")
                    for s in range(SPT):
                        nc.gpsimd.indirect_dma_start(
                            out=xs[:, s * din:(s + 1) * din], out_offset=None,
                            in_=x_tab[:, :],
                            in_offset=bass.IndirectOffsetOnAxis(
                                ap=idx_t[:, s:s + 1], axis=0))
                    ea_s = sup.tile([128, SPT * 9], F32, tag="eas")
                    nc.sync.dma_start(out=ea_s[:], in_=t_in['ea'][nt, :, :])
                    sw_s = sup.tile([128, SPT * 128], BF16, tag="sws")
                    nc.sync.dma_start(out=sw_s[:], in_=t_in['swin'][nt, :, :])
                    if not last:
                        w_s = sup.tile([128, SPT * do], F32, tag="ws")
                        nc.sync.dma_start(out=w_s[:], in_=t_in[f'w{li}'][nt, :, :])
                    for t in range(SPT):
                        ea_t = ea_s[:, t * 9:(t + 1) * 9]
                        xs_t = xs[:, t * din:(t + 1) * din]
                        z = zb.tile([128, nchunk * 128], BF16, tag="z")
                        if din > 128:
                            mul_bcast(z, 0, xs_t[:, 0:128], ea_t, 9, 128)
                            mul_bcast(z, 9 * 128, xs_t[:, 128:din], ea_t, 9,
                                      din - 128)
                        else:
                            mul_bcast(z, 0, xs_t[:, 0:din], ea_t, 9, din)
                        zT = ztp.tile([128, nchunk * 128], BF16, tag="zT")
                        transpose_to(zT, z, K)
                        msg = pm.tile([128, do], F32, space="PSUM", tag="msg")
                        for ci in range(nchunk):
                            r = min(128, K - ci * 128)
                            nc.tensor.matmul(
                                out=msg[:],
                                lhsT=zT[0:r, ci * 128:ci * 128 + 128],
                                rhs=(Wf_t[0:r, :] if last
                                     else Wz[li][0:r, ci * do:(ci + 1) * do]),
                                start=(ci == 0), stop=(ci == nchunk - 1))
                        mw = sm.tile([128, do], BF16, tag="mw")
                        if last:
                            nc.vector.tensor_copy(out=mw[:], in_=msg[:])
                        else:
                            nc.vector.tensor_tensor(
                                out=mw[:], in0=msg[:],
                                in1=w_s[:, t * do:(t + 1) * do],
                                op=mybir.AluOpType.mult)
                        sc_more = (not last) and li > 0
                        nc.tensor.matmul(
                            out=agg[:], lhsT=sw_s[:, t * 128:(t + 1) * 128],
                            rhs=mw[:], start=(t == 0),
                            stop=(t == SPT - 1 and not sc_more))
                    # ---- finalize node tile ----
                    if last:
                        a4 = sm.tile([128, 8], BF16, tag="a4")
                        nc.vector.tensor_copy(out=a4[:], in_=agg[:])
                        nc.tensor.matmul(
                            out=out_ps[:], lhsT=bgr_t[:, nt * 16:(nt + 1) * 16],
                            rhs=a4[:], start=(nt == 0), stop=(nt == NT - 1))
                        continue
                    xn = ndp.tile([128, do], F32, tag="xn")
                    if li == 0:
                        sct = ndp.tile([128, do], F32, tag="sct")
                        nc.sync.dma_start(
                            out=sct[:], in_=t_in['sc0'][nt * 128:(nt + 1) * 128, :])
                        nc.vector.tensor_tensor(out=xn[:], in0=agg[:], in1=sct[:],
                                                op=mybir.AluOpType.add)
                        nc.scalar.activation(
                            out=xn[:], in_=xn[:],
                            func=mybir.ActivationFunctionType.Silu)
                    else:
                        xloc_t = ndp.tile([128, din], F32, tag="xloc_t")
                        nc.sync.dma_start(
                            out=xloc_t[:],
                            in_=xloc[li - 1][nt * 128:(nt + 1) * 128, :])
                        Ksc = din * 4
                        nsck = -(-Ksc // 128)
                        zs = zb.tile([128, nsck * 128], BF16, tag="zsc")
                        na = nattr_t[:, nt * 4:(nt + 1) * 4]
                        if din > 128:
                            mul_bcast(zs, 0, xloc_t[:, 0:128], na, 4, 128)
                            mul_bcast(zs, 4 * 128, xloc_t[:, 128:din], na, 4,
                                      din - 128)
                        else:
                            mul_bcast(zs, 0, xloc_t[:, 0:din], na, 4, din)
                        zsT = ztp.tile([128, nsck * 128], BF16, tag="zscT")
                        transpose_to(zsT, zs, Ksc)
                        for ci in range(nsck):
                            r = min(128, Ksc - ci * 128)
                            nc.tensor.matmul(
                                out=agg[:],
                                lhsT=zsT[0:r, ci * 128:ci * 128 + 128],
                                rhs=Ws[li][0:r, ci * do:(ci + 1) * do],
                                start=False, stop=(ci == nsck - 1))
                        if li < 3:
                            nc.scalar.activation(
                                out=xn[:], in_=agg[:],
                                func=mybir.ActivationFunctionType.Silu)
                        else:
                            nc.scalar.copy(out=xn[:], in_=agg[:])
                    nc.sync.dma_start(out=xloc[li][nt * 128:(nt + 1) * 128, :],
                                      in_=xn[:])
                if not last:
                    nc.gpsimd.collective_compute(
                        "AllGather", mybir.AluOpType.bypass,
                        replica_groups=[list(range(N_CORES))],
                        ins=[xloc[li].opt()], outs=[xg[li].opt()])

            edge_layer(0, t_in['xpad0'])
            edge_layer(1, xg[0])
            edge_layer(2, xg[1])
            edge_layer(3, xg[2])
            edge_layer(4, xg[3], last=True)

            ot = sm.tile([16, 8], F32, tag="ot")
            nc.vector.tensor_copy(out=ot[:], in_=out_ps[:])
            nc.sync.dma_start(out=out[:, :], in_=ot[:])
    return out


def kernel(**inputs):
    node_input = np.asarray(inputs['node_input'], np.float32)
    node_attr = np.asarray(inputs['node_attr'], np.float32)
    edge_attr = np.asarray(inputs['edge_attr'], np.float32)
    emb = np.asarray(inputs['edge_length_embedding'], np.float32)
    params = inputs['params']
    edge_src = np.asarray(inputs['edge_src']).astype(np.int64)
    edge_dst = np.asarray(inputs['edge_dst']).astype(np.int64)
    batch = np.asarray(inputs['batch']).astype(np.int64)

    pk = _host_prep(node_input, node_attr, edge_attr, emb, params,
                    edge_src, edge_dst, batch)

    nc = bacc.Bacc("TRN2", target_bir_lowering=False, debug=False,
                   num_devices=N_CORES)
    _build(nc, pk['SPT'])
    nc.compile()

    in_maps = []
    for c in range(N_CORES):
        m = dict(idx=pk['idx'][c], ea=pk['ea'][c], swin=np.asarray(pk['swin'][c]),
                 Wf2d=np.asarray(pk['Wf2d']), xpad0=pk['xpad0'],
                 nattr=pk['nattr'][c], sc0=pk['sc0'][c],
                 bgr=np.asarray(pk['bgr'][c]))
        for li in range(4):
            m[f'w{li}'] = pk['w'][li][c]
            m[f'W2d{li}'] = np.asarray(pk['W2d'][li])
            m[f'Wsc{li}'] = np.asarray(pk['Wsc'][li])
        in_maps.append(m)

    import os, time as _time
    trace = os.environ.get("KERNEL_TRACE") == "1"
    _t0 = _time.time()
    res = bass_utils.run_bass_kernel_spmd(nc, in_maps,
                                          core_ids=list(range(N_CORES)),
                                          trace=trace)
    kernel.last_run_s = _time.time() - _t0
    out = np.zeros((16, 8), np.float32)
    for c in range(N_CORES):
        out += res.results[c]['out']
    kernel.last_results = res
    return out
